# revision 15
# baseline (speedup 1.0000x reference)
"""Bahdanau additive attention on 8 Trainium2 NeuronCores.

Reference computation (per batch b):
    q_proj = query[b] @ Wa_w.T + Wa_b                 # [1, H]
    k_proj = keys[b] @ Ua_w.T + Ua_b                  # [S, H]
    scores = tanh(q_proj + k_proj) @ Va_w.T (+ Va_b)  # [S, 1]
    weights = softmax(scores, axis=S)
    out[b]  = weights * values[b]                     # [S, H] outer product

Shapes: B=32, S=4096, H=512, fp32.  Sharding: batch across 8 cores (4 each).
Va_b is a scalar added to every score of a batch -> softmax-invariant -> dropped.

v2 dataflow (transposed-GEMM orientation, per 512-row s-block):
  DMA 1MB keys block -> PE transpose keys -> copy PSUM->SBUF (keysT, f32r)
  -> PE GEMM k_projT[o_chunk, s] = sum_h UaT2[h][o].T @ keysT[h]  (static f32r
     weights -> LDWEIGHTS prefetchable) -> ACT tanh with fused per-partition
     bias c[b][o_chunk] -> PE Va-dot (scores[1, s] accumulated over o_chunks).
Softmax per batch: PE mini-transposes scores row -> [128, 32] -> ACT exp with
  accum_out -> PE ones-matmul partition sum -> DVE reciprocal -> PE broadcast.
Output: DVE tensor_scalar(values_rep * w[s]) -> 1MB DMA out.

HW constraints baked in (found by bisection on this machine):
  - matmul lhsT (weights) APs must be whole contiguous tiles; strided slices
    of wider tiles hang (fp32) or crash (f32r) the exec unit.
  - operands of f32r matmuls must be f32r-typed tensors written by a rounding
    producer (ACT/DVE copy), per the BIR verifier.
  - tensor_tensor_reduce crashes; scalar_tensor_tensor(accum_out=) works.
"""

import sys

if "/opt/trn_rl_repo" not in sys.path:
    sys.path.insert(0, "/opt/trn_rl_repo")

import numpy as np

B, S, H = 32, 4096, 512
N_CORES = 8
BPC = B // N_CORES          # batches per core
P = 128                     # partitions
NTILES = S // P             # 32 s-tiles per batch
GRP = 4                     # s-tiles per DMA group / s-block (512 rows, 1MB)
NGRP = NTILES // GRP        # 8 s-blocks per batch
NCH = H // P                # 4 chunks of the H dimension
SB = GRP * P                # s-block size in rows (512)

_compiled = None


def _build():
    import concourse.bacc as bacc
    import concourse.mybir as mybir
    import concourse.tile as tile
    from concourse import masks

    dt = mybir.dt
    f32 = dt.float32
    f32r = dt.float32r
    bf16 = dt.bfloat16
    AF = mybir.ActivationFunctionType

    nc = bacc.Bacc("TRN2", target_bir_lowering=False, debug=False)

    keys_d = nc.dram_tensor("keys", [BPC, S, H], f32, kind="ExternalInput")
    query_d = nc.dram_tensor("query", [BPC, H], f32, kind="ExternalInput")
    values_d = nc.dram_tensor("values", [BPC, H], f32, kind="ExternalInput")
    wa_d = nc.dram_tensor("Wa_w", [H, H], f32, kind="ExternalInput")
    ua_d = nc.dram_tensor("Ua_w", [H, H], f32, kind="ExternalInput")
    va_d = nc.dram_tensor("Va_w", [1, H], f32, kind="ExternalInput")
    wab_d = nc.dram_tensor("Wa_b", [1, H], f32, kind="ExternalInput")
    uab_d = nc.dram_tensor("Ua_b", [1, H], f32, kind="ExternalInput")
    out_d = nc.dram_tensor("out", [BPC, S, H], f32, kind="ExternalOutput")

    with tile.TileContext(nc) as tc:
        with (
            tc.tile_pool(name="const", bufs=1) as cpool,
            tc.tile_pool(name="keys", bufs=3) as kpool,
            tc.tile_pool(name="kT", bufs=3) as ktpool,
            tc.tile_pool(name="tanh", bufs=2) as thpool,
            tc.tile_pool(name="outp", bufs=5) as opool,
            tc.tile_pool(name="batch", bufs=2) as bpool,
            tc.tile_pool(name="small", bufs=2) as spool,
            tc.tile_pool(name="ps_kT", bufs=2, space="PSUM") as ps_kt,
            tc.tile_pool(name="ps_mm", bufs=4, space="PSUM") as ps_mm,
            tc.tile_pool(name="ps_sm", bufs=2, space="PSUM") as ps_sm,
        ):
            # ---------- one-time prep ----------
            ident = cpool.tile([P, P], f32)
            masks.make_identity(nc, ident[:])
            identb = cpool.tile([P, P], bf16)
            nc.vector.tensor_copy(out=identb[:], in_=ident[:])
            ones_row = cpool.tile([1, P], f32)
            nc.gpsimd.memset(ones_row[:], 1.0)
            ones_col = cpool.tile([P, 1], f32)
            nc.gpsimd.memset(ones_col[:], 1.0)
            ones_4 = cpool.tile([1, BPC], f32r)
            ones_4_f = spool.tile([1, BPC], f32, tag="tmp1")
            nc.gpsimd.memset(ones_4_f[:], 1.0)
            nc.scalar.copy(ones_4[:], ones_4_f[:])

            # bias_sum = Wa_b + Ua_b -> 4 contiguous [1, P] f32r chunks
            wab_sb = spool.tile([1, H], f32, tag="bias_ld")
            uab_sb = spool.tile([1, H], f32, tag="bias_ld")
            nc.sync.dma_start(out=wab_sb[:], in_=wab_d.ap())
            nc.sync.dma_start(out=uab_sb[:], in_=uab_d.ap())
            bias_sum = spool.tile([1, H], f32, tag="bias_sum")
            nc.vector.tensor_add(out=bias_sum[:], in0=wab_sb[:], in1=uab_sb[:])
            bias_ch = []
            for c in range(NCH):
                t = cpool.tile([1, P], f32r, tag=f"bias{c}", name=f"bias{c}")
                nc.scalar.copy(t[:], bias_sum[:, c * P : (c + 1) * P])
                bias_ch.append(t)

            # Va as 4 contiguous [P, 1] f32r column chunks
            va_sb = spool.tile([1, H], f32, tag="va_ld")
            nc.sync.dma_start(out=va_sb[:], in_=va_d.ap())
            va_col = []
            for c in range(NCH):
                ps = ps_sm.tile([P, H], f32, tag="sm", name="vacol_ps")
                nc.tensor.transpose(
                    ps[:, :1], va_sb[:1, c * P : (c + 1) * P], ident[:1, :1]
                )
                t = cpool.tile([P, 1], f32r, tag=f"vac{c}", name=f"vac{c}")
                nc.scalar.copy(t[:], ps[:, :1])
                va_col.append(t)

            # UaT2 / WaT2: [H, H] (o, h) -> 16 contiguous [P, P] f32r tiles
            # dst[h_chunk][o_chunk][h_in, o_in] = W[o_chunk*P + o_in, h_chunk*P + h_in]
            def load_transposed(src_d, tag, dtype):
                dst = [
                    [
                        cpool.tile(
                            [P, P], dtype, tag=f"{tag}T{r}{c}", name=f"{tag}T{r}{c}"
                        )
                        for c in range(NCH)
                    ]
                    for r in range(NCH)
                ]
                for c in range(NCH):  # o chunk
                    src = spool.tile([P, H], f32, tag="wload")
                    nc.sync.dma_start(
                        out=src[:], in_=src_d.ap()[c * P : (c + 1) * P, :]
                    )
                    for r in range(NCH):  # h chunk
                        ps = ps_sm.tile([P, H], f32, tag="sm", name="wtr_ps")
                        nc.tensor.transpose(
                            ps[:, :P], src[:, r * P : (r + 1) * P], ident[:]
                        )
                        nc.scalar.copy(dst[r][c][:], ps[:, :P])
                return dst

            uaT = load_transposed(ua_d, "ua", f32r)
            waT = load_transposed(wa_d, "wa", f32r)

            # qT: 4 chunks [P, BPC] f32r
            q_sb = spool.tile([BPC, H], f32, tag="qload")
            nc.sync.dma_start(out=q_sb[:], in_=query_d.ap())
            qT = []
            for c in range(NCH):
                ps = ps_sm.tile([P, H], f32, tag="sm", name="qtr_ps")
                nc.tensor.transpose(
                    ps[:, :BPC], q_sb[:, c * P : (c + 1) * P], ident[:BPC, :BPC]
                )
                t = cpool.tile([P, BPC], f32r, tag=f"qT{c}", name=f"qT{c}")
                nc.scalar.copy(t[:], ps[:, :BPC])
                qT.append(t)

            # c_col[b][o_chunk] [P, 1] f32: query[b] @ Wa.T + (Wa_b + Ua_b),
            # computed as [o_chunk, batch] = sum_h WaT2[h][o].T @ qT[h] + bias
            c_col = [[None] * NCH for _ in range(BPC)]
            for o in range(NCH):
                ps = ps_sm.tile([P, H], f32, tag="sm", name="c_ps")
                for r in range(NCH):
                    nc.tensor.matmul(
                        ps[:, :BPC],
                        waT[r][o][:],
                        qT[r][:],
                        start=(r == 0),
                        stop=False,
                    )
                nc.tensor.matmul(
                    ps[:, :BPC], bias_ch[o][:], ones_4[:], start=False, stop=True
                )
                for b in range(BPC):
                    t = cpool.tile([P, 1], f32, tag=f"c{b}_{o}", name=f"c{b}_{o}")
                    nc.scalar.copy(t[:], ps[:, b : b + 1])
                    c_col[b][o] = t

            # ---------- main loop (software-pipelined emission:
            # phase2 of batch b-1 interleaves with phase1 of batch b) ----------
            prev = None  # (w_sb, v_rep, out_grp) of previous batch

            def phase2_group(state, g):
                w_p, vrep_p, outgrp_p = state
                o4 = opool.tile([P, GRP * H], f32, name="o4")
                for u in range(GRP):
                    t_idx = g * GRP + u
                    nc.vector.tensor_scalar_mul(
                        o4[:, u * H : (u + 1) * H],
                        vrep_p[:],
                        w_p[:, t_idx : t_idx + 1],
                    )
                nc.sync.dma_start(
                    out=outgrp_p[g],
                    in_=o4[:].rearrange("p (u h) -> p u h", u=GRP),
                )

            for b in range(BPC + 1):
                if b < BPC:
                    # values[b] replicated across partitions (fp32 ones-matmul)
                    v_sb = spool.tile([1, H], f32, tag="vload")
                    nc.sync.dma_start(out=v_sb[:], in_=values_d.ap()[b : b + 1, :])
                    vr_ps = ps_sm.tile([P, H], f32, tag="sm", name="vrep_ps")
                    nc.tensor.matmul(
                        vr_ps[:], ones_row[:], v_sb[:], start=True, stop=True
                    )
                    v_rep = bpool.tile([P, H], f32, tag="vrep")
                    nc.scalar.copy(v_rep[:], vr_ps[:])

                    scores_row = bpool.tile([1, S], f32, tag="scores_row")
                    keys_grp = keys_d.ap()[b].rearrange(
                        "(g u p) h -> g p u h", u=GRP, p=P
                    )

                for g0 in range(0, NGRP, 2):
                    pair = (g0, g0 + 1)
                    if b < BPC:
                        # ---- phase 1: paired s-blocks (b, g0) (b, g0+1) ----
                        kt_pair = {}
                        for g in pair:
                            k4 = kpool.tile([P, GRP * H], f32, name="k4")
                            nc.sync.dma_start(
                                out=k4[:].rearrange("p (u h) -> p u h", u=GRP),
                                in_=keys_grp[g],
                            )
                            kt_sb = []
                            for c in range(NCH):
                                kt_ps = ps_kt.tile(
                                    [P, SB], f32, tag="ktr", name="kt_ps"
                                )
                                for u in range(GRP):
                                    nc.tensor.transpose(
                                        kt_ps[:, u * P : (u + 1) * P],
                                        k4[:, u * H + c * P : u * H + (c + 1) * P],
                                        ident[:],
                                    )
                                t = ktpool.tile(
                                    [P, SB], f32r, tag=f"kt{c}", name=f"kt{c}"
                                )
                                if c == 0:
                                    nc.scalar.copy(t[:], kt_ps[:])
                                else:
                                    nc.vector.tensor_copy(out=t[:], in_=kt_ps[:])
                                kt_sb.append(t)
                            kt_pair[g] = kt_sb
                        # GEMM: same weights serve both blocks back-to-back
                        mm_pair = {g: [] for g in pair}
                        for o in range(NCH):
                            for g in pair:
                                mm_pair[g].append(
                                    ps_mm.tile([P, SB], f32, tag="gemm", name="mm_ps")
                                )
                            for c in range(NCH):
                                for g in pair:
                                    nc.tensor.matmul(
                                        mm_pair[g][o][:],
                                        uaT[c][o][:],
                                        kt_pair[g][c][:],
                                        start=(c == 0),
                                        stop=(c == NCH - 1),
                                    )
                        th_pair = {g: [] for g in pair}
                        for g in pair:
                            for o in range(NCH):
                                th = thpool.tile(
                                    [P, SB], f32r, tag=f"th{o}", name=f"th{o}"
                                )
                                nc.scalar.activation(
                                    th[:], mm_pair[g][o][:], AF.Tanh,
                                    bias=c_col[b][o][:], scale=1.0,
                                )
                                th_pair[g].append(th)
                        sc_pair = {
                            g: ps_sm.tile([P, H], f32, tag="sm", name="sc_ps")
                            for g in pair
                        }
                        for o in range(NCH):
                            for g in pair:
                                nc.tensor.matmul(
                                    sc_pair[g][:1, :],
                                    va_col[o][:],
                                    th_pair[g][o][:],
                                    start=(o == 0),
                                    stop=(o == NCH - 1),
                                )
                        for g in pair:
                            nc.scalar.copy(
                                scores_row[:1, g * SB : (g + 1) * SB],
                                sc_pair[g][:1, :],
                            )
                    # ---- phase 2 groups (b-1, g0/g0+1) ----
                    if prev is not None:
                        phase2_group(prev, g0)
                        phase2_group(prev, g0 + 1)

                if prev is not None:
                    prev = None
                if b == BPC:
                    break

                # ---- softmax for batch b ----
                sct_ps = ps_sm.tile([P, H], f32, tag="sm", name="sct_ps")
                for t_idx in range(NTILES):
                    nc.tensor.transpose(
                        sct_ps[:, t_idx : t_idx + 1],
                        scores_row[:1, t_idx * P : (t_idx + 1) * P],
                        ident[:1, :1],
                    )
                scores_sb = bpool.tile([P, NTILES], f32, tag="scores_sb")
                nc.scalar.copy(scores_sb[:], sct_ps[:, :NTILES])
                w_sb = bpool.tile([P, NTILES], f32, tag="wts")
                partials = spool.tile([P, 1], f32, tag="partials")
                nc.scalar.activation(
                    w_sb[:], scores_sb[:], AF.Exp, accum_out=partials[:]
                )
                tot_ps_t = ps_sm.tile([P, H], f32, tag="sm", name="tot_ps")
                nc.tensor.matmul(
                    tot_ps_t[:1, :1], partials[:], ones_col[:], start=True, stop=True
                )
                tot_sb = spool.tile([1, 1], f32, tag="tot_sb")
                nc.scalar.copy(tot_sb[:], tot_ps_t[:1, :1])
                inv_sb = spool.tile([1, 1], f32, tag="inv_sb")
                nc.vector.reciprocal(inv_sb[:], tot_sb[:])
                invr_ps_t = ps_sm.tile([P, H], f32, tag="sm", name="invr_ps")
                nc.tensor.matmul(
                    invr_ps_t[:, :1], ones_row[:], inv_sb[:], start=True, stop=True
                )
                invr_sb = spool.tile([P, 1], f32, tag="invr_sb")
                nc.scalar.copy(invr_sb[:], invr_ps_t[:, :1])
                nc.vector.tensor_scalar_mul(w_sb[:], w_sb[:], invr_sb[:])

                out_grp = out_d.ap()[b].rearrange(
                    "(g u p) h -> g p u h", u=GRP, p=P
                )
                prev = (w_sb, v_rep, out_grp)

    nc.compile()
    return nc


def _get_compiled():
    global _compiled
    if _compiled is None:
        _compiled = _build()
    return _compiled


def _make_in_maps(inputs):
    query = np.ascontiguousarray(inputs["query"], dtype=np.float32).reshape(B, H)
    keys = np.ascontiguousarray(inputs["keys"], dtype=np.float32)
    values = np.ascontiguousarray(inputs["values"], dtype=np.float32).reshape(B, H)
    wa_w = np.ascontiguousarray(inputs["Wa_w"], dtype=np.float32)
    ua_w = np.ascontiguousarray(inputs["Ua_w"], dtype=np.float32)
    va_w = np.ascontiguousarray(inputs["Va_w"], dtype=np.float32).reshape(1, H)
    wa_b = np.ascontiguousarray(inputs["Wa_b"], dtype=np.float32).reshape(1, H)
    ua_b = np.ascontiguousarray(inputs["Ua_b"], dtype=np.float32).reshape(1, H)
    in_maps = []
    for c in range(N_CORES):
        sl = slice(c * BPC, (c + 1) * BPC)
        in_maps.append(
            {
                "keys": keys[sl],
                "query": query[sl],
                "values": values[sl],
                "Wa_w": wa_w,
                "Ua_w": ua_w,
                "Va_w": va_w,
                "Wa_b": wa_b,
                "Ua_b": ua_b,
            }
        )
    return in_maps


def kernel(**inputs) -> np.ndarray:
    from concourse import bass_utils

    nc = _get_compiled()
    res = bass_utils.run_bass_kernel_spmd(
        nc, _make_in_maps(inputs), core_ids=list(range(N_CORES)), trace=False
    )
    out = np.concatenate([res.results[c]["out"] for c in range(N_CORES)], axis=0)
    return out.reshape(B, S, H)


def run_traced(inputs):
    """test.py helper: run with NTFF profiling, return (output, BassKernelResults)."""
    from concourse import bass_utils

    nc = _get_compiled()
    res = bass_utils.run_bass_kernel_spmd(
        nc, _make_in_maps(inputs), core_ids=list(range(N_CORES)), trace=True
    )
    out = np.concatenate([res.results[c]["out"] for c in range(N_CORES)], axis=0)
    return out.reshape(B, S, H), res
